# revision 36
# baseline (speedup 1.0000x reference)
# Bass/Tile Trainium2 kernel for batched multi-head causal self-attention.
#
# Problem: x[B=2,T=2048,C=1024], 16 heads (hd=64), causal softmax attention,
# output projection. Full (unsharded) inputs in, full output out.
#
# Sharding (Megatron-style): 8 cores = 2 batch groups x 4 head groups.
# Core i handles batch b = i // 4 and heads [4*(i%4) : 4*(i%4)+4).
# Each core computes Q/K/V projections for its 4 heads, causal attention,
# and a partial output projection (contribution of its heads).  The host
# sums the 4 partials per batch (the Megatron all-reduce) and adds bias.
#
# v2 design notes (vs the phase-serial v1):
#   - Everything kept "transposed" (feature dim on partitions): xT [C, T],
#     QT/KT [64, T] per head, heads packed in pairs on the 128 partitions
#     (even head at [0:64], odd head at [64:128]).
#   - Attention runs per (head-pair, q-block, k-tile): the two K=64 S^T
#     matmuls of a pair are issued back-to-back on PE row tiles T0/T8
#     (tile_position (0,0)/(64,0)) into bank-distinct halves of one merged
#     PSUM tile, so the hardware can stream them concurrently; ONE merged
#     EXP per k-tile covers both heads.  On diagonal k-tiles both the S^T
#     matmuls and the EXP are column-restricted to the causal region.
#   - V' = [V | ones] (65 columns) so P@V' also yields the softmax
#     denominator in row 64: no partition reductions anywhere.
#   - Normalization is DMA-free: reciprocal_approx_fast on the denominator
#     row (partition 64), then two rank-1 PE matmuls (K=1 ones lhsT at
#     base partition 64) broadcast the reciprocals into PSUM, and DVE
#     multiplies produce the normalized O^T.
#   - QKV projection chains (one PSUM bank each) and output-projection
#     chunks are interleaved into the attention stream as elastic PE
#     filler, so PE never idles while ACT grinds through the exps.
#   - Partial outputs are written as fp16 (halves the output DMA); the
#     host sums the 4 partials per batch in fp32 and adds the bias.
#   - PSUM budget (8 banks): sg 2x2 + po 1x2 + mm1 2x1 = 8.

import numpy as np

import concourse.bass as bass
import concourse.tile as tile
from concourse import bacc, mybir
from concourse import bass_utils

F32 = mybir.dt.float32
F32R = mybir.dt.float32r
F16 = mybir.dt.float16
BF16 = mybir.dt.bfloat16
ATT_DT = BF16   # dtype of attention operands (qt/kt/v'/pt/mask)

B, T, C, H = 2, 2048, 1024, 16
HD = C // H            # 64 head dim
NCORES = 8
HPC = 4                # heads per core
DSEL = HPC * HD        # 256 feature dims per core
NTT = T // 128         # 16 t-tiles of 128
NTB = T // 512         # 4 t-blocks of 512
NCC = C // 128         # 8 c-chunks of 128
NQB = T // 512         # 4 q-blocks of 512


def build_program():
    nc = bacc.Bacc("TRN2", target_bir_lowering=False, debug=False)

    # host-prepared "SBUF images": [128 partitions, ...] with long
    # contiguous per-partition lines for efficient DMA.
    # xT layout is t-block-major: [128, tb, cc, 512] -> 8KB lines per tb.
    xT = nc.dram_tensor("xT", [128, NTB, NCC, 512], BF16, kind="ExternalInput").ap()
    wqT = nc.dram_tensor("wqT", [128, NCC * DSEL], BF16, kind="ExternalInput").ap()
    wkT = nc.dram_tensor("wkT", [128, NCC * DSEL], BF16, kind="ExternalInput").ap()
    wvT = nc.dram_tensor("wvT", [128, NCC * DSEL], BF16, kind="ExternalInput").ap()
    wpT = nc.dram_tensor("wpT", [128, 2 * C], F32R, kind="ExternalInput").ap()
    maskd = nc.dram_tensor("maskd", [128, 2, 128], ATT_DT, kind="ExternalInput").ap()
    out_p = nc.dram_tensor("out_p", [T, C], F16, kind="ExternalOutput").ap()

    scale = 1.0 / float(np.sqrt(HD))

    with tile.TileContext(nc) as tc:
        with (
            tc.tile_pool(name="consts", bufs=1) as consts,
            tc.tile_pool(name="persist", bufs=1) as persist,
            tc.tile_pool(name="xin", bufs=3) as xin,
            tc.tile_pool(name="pt", bufs=4) as ptpool,
            tc.tile_pool(name="norm", bufs=2) as norm,
            tc.tile_pool(name="outst", bufs=3) as outst,
            tc.tile_pool(name="psum", bufs=1, space="PSUM") as pa,
        ):
            # ---- constants / weights -------------------------------------
            wq_sb = consts.tile([128, NCC, DSEL], BF16, tag="wq")
            wk_sb = consts.tile([128, NCC, DSEL], BF16, tag="wk")
            wv_sb = consts.tile([128, NCC, DSEL], BF16, tag="wv")
            wp_sb = consts.tile([128, 2, C], F32R, tag="wp")
            mk_sb = consts.tile([128, 2, 128], ATT_DT, tag="mk")
            ones_sb = consts.tile([128, 64], F32, tag="ones")
            warm_sb = consts.tile([128, 16], F32, tag="warm")

            # one descriptor per tensor: DMA-issue time on the sequencers
            # (~0.7us each) dominates the prologue, not DMA bandwidth
            nc.sync.dma_start(
                out=wq_sb[:].rearrange("p cc d -> p (cc d)"), in_=wqT)
            # ACT exp table load happens off the critical path, right away
            nc.vector.memset(ones_sb[:], 1.0)
            nc.scalar.activation(
                out=warm_sb[:], in_=ones_sb[:, 0:16],
                func=mybir.ActivationFunctionType.Exp, scale=0.1)
            # wk/wv issued on sync AFTER xt(tb0) (see top-level schedule) so
            # the DMA engines stream tensors in consumption order

            # ---- persistent activations ----------------------------------
            qt_sb = persist.tile([128, 2, T], ATT_DT, tag="qt")
            kt_sb = persist.tile([128, 2, T], ATT_DT, tag="kt")
            ot_sb = persist.tile([128, 2, T], F32R, tag="ot")
            # V' per k-tile: 4 heads x (64 V cols + 1 ones col)
            v_sb = persist.tile([128, NTT, HPC * (HD + 1)], ATT_DT, tag="v")
            for h in range(HPC):
                nc.vector.tensor_copy(
                    out=v_sb[:, :, h * 65 + 64 : h * 65 + 65],
                    in_=ones_sb[:, 0:NTT].rearrange("p (t o) -> p t o", o=1),
                )

            # ---- QKV projection chains (one PSUM bank each) --------------
            xts = [None] * NTB

            def emit_xt_dma(tb):
                xt = xin.tile([128, NCC, 512], BF16, tag="xt", name=f"xt{tb}")
                for ch in range(2):     # cc-half split: chains start earlier
                    ccs = slice(ch * 4, ch * 4 + 4)
                    nc.sync.dma_start(
                        out=xt[:, ccs].rearrange("p cc t -> p (cc t)"),
                        in_=xT[:, tb, ccs].rearrange("p cc t -> p (cc t)"))
                xts[tb] = xt

            def qk_chain(tb, which, h2):
                # QT/KT[d-half h2, t-block tb]; generator yields every 2 MMs
                w_sb = wq_sb if which == 0 else wk_sb
                dst = qt_sb if which == 0 else kt_sb
                xt = xts[tb]
                pq = pa.tile([128, 512], F32, tag="mm1", bufs=2, name="pq")
                for cc in range(NCC):
                    nc.tensor.matmul(
                        pq[:], w_sb[:, cc, h2 * 128 : h2 * 128 + 128],
                        xt[:, cc, :],
                        start=(cc == 0), stop=(cc == NCC - 1))
                    if cc % 2 == 1:
                        yield
                nc.vector.tensor_copy(
                    out=dst[:, h2, tb * 512 : tb * 512 + 512], in_=pq[:])

            def v_chain(tb, ct):
                # V' rows for t-tiles (4*tb + 2*ct, +1), 256 d cols each
                xt = xts[tb]
                pv = pa.tile([128, 512], F32, tag="mm1", bufs=2, name="pv")
                for cc in range(NCC):
                    for m in range(2):
                        # both halves share one PSUM bank: only the first
                        # toucher may set start, only the last may set stop
                        nc.tensor.matmul(
                            pv[:, m * 256 : m * 256 + 256],
                            xt[:, cc, (2 * ct + m) * 128 : (2 * ct + m) * 128 + 128],
                            wv_sb[:, cc, :],
                            start=(cc == 0 and m == 0),
                            stop=(cc == NCC - 1 and m == 1))
                    if cc % 2 == 1:
                        yield
                tt0 = 4 * tb + 2 * ct
                # [128, m2, h4, d64] -> v_sb[:, tt0:tt0+2, h*65:h*65+64]
                nc.vector.tensor_copy(
                    out=v_sb[:, tt0 : tt0 + 2, :].rearrange(
                        "p m (h e) -> p m h e", h=HPC)[:, :, :, 0:64],
                    in_=pv[:].rearrange("p (m h e) -> p m h e", m=2, h=HPC),
                )

            def qkv_chains(tb):
                return ([qk_chain(tb, 0, h2) for h2 in range(2)]
                        + [qk_chain(tb, 1, h2) for h2 in range(2)]
                        + [v_chain(tb, ct) for ct in range(2)])

            # ---- output projection chunks --------------------------------
            dma_engs = [nc.gpsimd]

            def proj_tt(tt, di):
                # generator: yields after each 2-MM projection chunk
                ob = outst.tile([128, 1024], F16, tag="ob", name="ob")
                for cb in range(2):
                    pc = pa.tile([128, 512], F32, tag="mm1", bufs=2, name="pc")
                    for hpp in range(2):
                        nc.tensor.matmul(
                            pc[:],
                            ot_sb[:, hpp, tt * 128 : tt * 128 + 128],
                            wp_sb[:, hpp, cb * 512 : cb * 512 + 512],
                            start=(hpp == 0), stop=(hpp == 1))
                    nc.vector.tensor_copy(out=ob[:, cb * 512 : cb * 512 + 512],
                                          in_=pc[:])
                    yield
                eng = dma_engs[di % len(dma_engs)]
                eng.dma_start(out=out_p[tt * 128 : tt * 128 + 128, :], in_=ob[:])

            # ---- attention -----------------------------------------------
            # filler: queues of generators emitting ~2 PE matmuls per step;
            # QKV for the next t-block has priority over deferred projection
            qkv_fill = []
            proj_fill = []

            def filler_step():
                for q in (qkv_fill, proj_fill):
                    while q:
                        try:
                            next(q[0])
                            return True
                        except StopIteration:
                            q.pop(0)
                return False

            def drain_gens(gens):
                for g in gens:
                    for _ in g:
                        pass

            def attn_chain(hp, qb):
                n_kt = 4 * (qb + 1)
                qs0 = qb * 512
                po = pa.tile([128, 1024], F32, tag="po", bufs=1, name="po")
                # diagonal k-tiles first: their masks (GpSimd) run early so
                # the steady-state flow is a pure PE->ACT->PE pipeline
                kt_order = list(range(4 * qb, n_kt)) + list(range(0, 4 * qb))
                for ki, kt in enumerate(kt_order):
                    j = kt - 4 * qb          # >= 0 on diagonal tiles
                    roff = 128 * j if j >= 0 else 0
                    sg = pa.tile([128, 1024], F32, tag="sg", bufs=2, name="sg")
                    pt = ptpool.tile([128, 1024], ATT_DT, tag="pt", name="pt")
                    for s in range(2):       # row tiles T0 / T8 back-to-back
                        psl = slice(64 * s, 64 * s + 64)
                        nc.tensor.matmul(
                            sg[:, s * 512 + roff : s * 512 + 512],
                            kt_sb[psl, hp, kt * 128 : kt * 128 + 128],
                            qt_sb[psl, hp, qs0 + roff : qs0 + 512],
                            start=True, stop=True,
                            tile_position=(64 * s, 0))
                    # one EXP for both heads, causal-column-restricted
                    sg3 = sg[:].rearrange("p (s q) -> p s q", s=2)
                    pt3 = pt[:].rearrange("p (s q) -> p s q", s=2)
                    nc.scalar.activation(
                        out=pt3[:, :, roff:512], in_=sg3[:, :, roff:512],
                        func=mybir.ActivationFunctionType.Exp, scale=scale)
                    if j >= 0:
                        # triangular mask on the diagonal 128-col block
                        nc.vector.tensor_mul(
                            pt3[:, :, roff : roff + 128],
                            pt3[:, :, roff : roff + 128],
                            mk_sb[:],
                        )
                    for s in range(2):
                        h = 2 * hp + s
                        nc.tensor.matmul(
                            po[0:65, s * 512 + roff : s * 512 + 512],
                            v_sb[:, kt, h * 65 : h * 65 + 65],
                            pt[:, s * 512 + roff : s * 512 + 512],
                            start=(ki == 0), stop=(ki == n_kt - 1))
                    filler_step()

                # ---- normalization (DMA-free) ----------------------------
                # den row -> partition 0 straight from PSUM (single-input
                # copies may shift partitions; broadcast reads absolute
                # partition 0); O rows evacuated in parallel
                dn = norm.tile([1, 1024], F32, tag="dn", name="dn")
                nc.vector.tensor_copy(out=dn[0:1, :], in_=po[64:65, :])
                ps = norm.tile([65, 1024], F32, tag="ps", name=f"ps{hp}{qb}")
                nc.vector.tensor_copy(out=ps[0:64, :], in_=po[0:64, :])
                rec = norm.tile([1, 1024], F32, tag="rec", name="rec")
                nc.vector.reciprocal_approx_fast(
                    out=rec[0:1, :], in_=dn[0:1, :])
                rb = norm.tile([128, 1024], F32, tag="rb", name="rb")
                for s in range(2):
                    nc.gpsimd.partition_broadcast(
                        rb[0:64, s * 512 : s * 512 + 512],
                        rec[0:1, s * 512 : s * 512 + 512],
                        channels=64)
                for s in range(2):
                    nc.vector.tensor_mul(
                        ot_sb[64 * s : 64 * s + 64, hp, qs0 : qs0 + 512],
                        ps[0:64, s * 512 : s * 512 + 512],
                        rb[0:64, s * 512 : s * 512 + 512],
                    )

            # ---- top-level schedule --------------------------------------
            emit_xt_dma(0)
            nc.sync.dma_start(
                out=wk_sb[:].rearrange("p cc d -> p (cc d)"), in_=wkT)
            nc.sync.dma_start(
                out=wv_sb[:].rearrange("p cc d -> p (cc d)"), in_=wvT)
            emit_xt_dma(1)
            nc.scalar.dma_start(out=mk_sb[:].rearrange("p s m -> p (s m)"),
                                in_=maskd.rearrange("p s m -> p (s m)"))
            nc.scalar.dma_start(
                out=wp_sb[:].rearrange("p h c -> p (h c)"), in_=wpT)
            # dummy matmuls keep PE busy through the DMA prologue so the
            # HAM clock-gate is already released (2.4 GHz) for real work
            nc.vector.memset(qt_sb[:, 0, 0:512], 1.0)
            for wi in range(16):
                pwarm = pa.tile([128, 512], F32, tag="mm1", bufs=2, name="pwarm")
                for _ in range(2):
                    nc.tensor.matmul(pwarm[:], qt_sb[:, 0, 0:128],
                                     qt_sb[:, 0, 0:512], start=True, stop=True)
            drain_gens(qkv_chains(0))
            emit_xt_dma(2)   # xin bufs=3: lands in a fresh slot

            qkv_fill.extend(qkv_chains(1))
            proj_di = [0]
            for qb in range(NQB):
                if qb > 0:
                    # attention for qb needs all QKV of t-block qb emitted
                    # first (the PE queue is in-order); deferred projection
                    # work stays queued — it has no such deadline
                    drain_gens(qkv_fill)
                    qkv_fill.clear()
                    if qb == 1:
                        emit_xt_dma(3)   # reuses xt0's slot (tb0 done)
                    if qb < NQB - 1:
                        qkv_fill.extend(qkv_chains(qb + 1))
                for hp in range(2):
                    attn_chain(hp, qb)
                # projection for this q-block becomes deferred filler work
                for tt in range(4 * qb, 4 * qb + 4):
                    di = proj_di[0]
                    proj_di[0] += 1
                    proj_fill.append(proj_tt(tt, di))
            drain_gens(qkv_fill)
            drain_gens(proj_fill)

    nc.compile()
    return nc


_NC_CACHE = None


def _get_program():
    global _NC_CACHE
    if _NC_CACHE is None:
        _NC_CACHE = build_program()
    return _NC_CACHE


def make_in_maps(x, Wq, Wk, Wv, Wp):
    import ml_dtypes
    x = np.asarray(x, np.float32)
    Wq = np.asarray(Wq, np.float32)
    Wk = np.asarray(Wk, np.float32)
    Wv = np.asarray(Wv, np.float32)
    Wp = np.asarray(Wp, np.float32)
    mk = np.triu(np.ones((128, 128), np.float32))  # mask[k,q] = (k <= q)
    maskd = np.stack([mk, mk], axis=1).astype(ml_dtypes.bfloat16)  # [128,2,128]
    in_maps = []
    for core in range(NCORES):
        b, hg = core // 4, core % 4
        sel = slice(hg * DSEL, (hg + 1) * DSEL)
        # xT image: feature c -> (cc, p) on partitions, t-block-major free dim
        xi = (x[b].T.reshape(NCC, 128, NTB, 512).transpose(1, 2, 0, 3))  # [128,tb,cc,512]
        wqi = Wq[sel, :].T.reshape(NCC, 128, DSEL).transpose(1, 0, 2).reshape(128, NCC * DSEL)
        wki = Wk[sel, :].T.reshape(NCC, 128, DSEL).transpose(1, 0, 2).reshape(128, NCC * DSEL)
        wvi = Wv[sel, :].T.reshape(NCC, 128, DSEL).transpose(1, 0, 2).reshape(128, NCC * DSEL)
        wpi = Wp[:, sel].T.reshape(2, 128, C).transpose(1, 0, 2).reshape(128, 2 * C)
        in_maps.append({
            "xT": np.ascontiguousarray(xi.astype(ml_dtypes.bfloat16)),
            "wqT": np.ascontiguousarray(wqi.astype(ml_dtypes.bfloat16)),
            "wkT": np.ascontiguousarray(wki.astype(ml_dtypes.bfloat16)),
            "wvT": np.ascontiguousarray(wvi.astype(ml_dtypes.bfloat16)),
            "wpT": np.ascontiguousarray(wpi),
            "maskd": np.ascontiguousarray(maskd),
        })
    return in_maps


def combine_outputs(results, bp):
    parts = [np.asarray(results[i]["out_p"], np.float32) for i in range(NCORES)]
    out = np.stack([
        parts[0] + parts[1] + parts[2] + parts[3],
        parts[4] + parts[5] + parts[6] + parts[7],
    ])
    return (out + np.asarray(bp, np.float32)).astype(np.float32)


def kernel(x, Wq, Wk, Wv, Wp, bp):
    nc = _get_program()
    in_maps = make_in_maps(x, Wq, Wk, Wv, Wp)
    res = bass_utils.run_bass_kernel_spmd(nc, in_maps, core_ids=list(range(NCORES)))
    return combine_outputs(res.results, bp)
